# revision 21
# baseline (speedup 1.0000x reference)
# Fused attention block (LeViT-style) for Trainium2, 8 NeuronCores, data-parallel over batch.
#
# reference computation (B=16, N=784, DIM=512, H=8, KD=64, VD=256):
#   qkv = BN(x @ qkv_w.T); split q,k,v per head
#   attn = softmax(q @ k.T * KD**-0.5 + attention_biases[:, bias_idxs])
#   out  = BN(silu(attn @ v reshaped) @ proj_w.T)
#
# Strategy (v2 — transposed-AV formulation):
#  - batch-parallel: 2 batches per core, weights/bias tables replicated, no collectives
#  - BN folded into weights on host; softmax scale folded into q weights
#  - scores computed transposed (S^T[j,i]); bias table is symmetric so bias adds unchanged
#  - unstabilized softmax (scores empirically bounded ~|10|, exp safe in fp32)
#  - heads in even/odd pairs at PE row bases 0/64: the K=64 score matmuls of the
#    two heads share the 128-wide xbus and execute concurrently
#  - AV computed TRANSPOSED: avT[d,i] = V^T @ P^T with lhsT=V (weight-stationary,
#    free dim 512/272) -> result lands in the exact lhsT layout proj needs, so
#    no PE transposes and no PSUM->SBUF repack copies
#  - softmax denominators via an all-ones lhsT matmul chain accumulating over j
#    chunks; output is replicated across all 128 partitions -> normalization is
#    a single DVE multiply per (head, d-half), no broadcast step
#  - v bias passes through softmax unchanged (attn rows sum to 1), so it is
#    dropped from the v GEMM and folded into silu's per-partition bias operand
#  - two generic 2-bank PSUM pools serve every phase (scores/den/av/qk/v/proj)

import numpy as np
import ml_dtypes

B, N, DIM = 16, 784, 512
H, KD, VD = 8, 64, 256
RES = 28
EPS = 1e-5
SCALE = KD ** -0.5
NCORES = 8
BL = B // NCORES          # batches per core
NJP = 896                 # padded j extent (7 * 128)

# t/j chunking over N=784: six 128-chunks + one 16-chunk
CHUNKS = [(i * 128, min(128, N - i * 128)) for i in range((N + 127) // 128)]
ITILES = [(0, 512), (512, N - 512)]   # free-dim tiles for 784 (<=512 per PSUM bank)

_CACHE = {}


def _build_nc():
    from contextlib import ExitStack
    import concourse.bacc as bacc
    import concourse.tile as tile
    from concourse import mybir

    bf = mybir.dt.bfloat16
    f32 = mybir.dt.float32
    AF = mybir.ActivationFunctionType
    MULT = mybir.AluOpType.mult

    nc = bacc.Bacc("TRN2", target_bir_lowering=False, debug=False)

    xT = nc.dram_tensor("xT", [BL, 128, 4, N], bf, kind="ExternalInput").ap()
    wqk = nc.dram_tensor("wqk", [128, 4, 1024], bf, kind="ExternalInput").ap()
    wv = nc.dram_tensor("wv", [128, 4, H * VD], bf, kind="ExternalInput").ap()
    wp = nc.dram_tensor("wp", [128, 16, DIM], bf, kind="ExternalInput").ap()
    bqk = nc.dram_tensor("bqk", [128, 8], f32, kind="ExternalInput").ap()
    bvT = nc.dram_tensor("bvT", [128, 16], f32, kind="ExternalInput").ap()
    bp = nc.dram_tensor("bp", [1, DIM], bf, kind="ExternalInput").ap()
    biast = nc.dram_tensor("biast", [H, 128, 7, N], bf, kind="ExternalInput").ap()
    ones2d = nc.dram_tensor("ones2d", [128, 128], bf, kind="ExternalInput").ap()
    out = nc.dram_tensor("out", [BL, N, DIM], f32, kind="ExternalOutput").ap()

    import concourse.bass as bass

    with ExitStack() as ctx:
        tc = ctx.enter_context(tile.TileContext(nc))
        consts = ctx.enter_context(tc.tile_pool(name="consts", bufs=1))
        xpool = ctx.enter_context(tc.tile_pool(name="xpool", bufs=2))
        qkpool = ctx.enter_context(tc.tile_pool(name="qkpool", bufs=1))
        vpool = ctx.enter_context(tc.tile_pool(name="vpool", bufs=1))
        silupool = ctx.enter_context(tc.tile_pool(name="silupool", bufs=1))
        biaspool = ctx.enter_context(tc.tile_pool(name="biaspool", bufs=2))
        ppool = ctx.enter_context(tc.tile_pool(name="ppool", bufs=2))
        smalls = ctx.enter_context(tc.tile_pool(name="smalls", bufs=2))
        gatepool = ctx.enter_context(tc.tile_pool(name="gatepool", bufs=2))
        fopool = ctx.enter_context(tc.tile_pool(name="fopool", bufs=2))
        # scores PSUM: one 4-bank tile (even head at 0, odd head at 1024) so
        # the even/odd row-group-paired matmuls are never skewed by buffer
        # recycling; psB: 2 x 2-bank tiles for den/av/v accumulators
        psA = ctx.enter_context(tc.tile_pool(name="psA", bufs=1, space="PSUM"))
        psB = ctx.enter_context(tc.tile_pool(name="psB", bufs=2, space="PSUM"))

        # ---- constants (spread across engine DGE queues so the startup
        #      critical path is max, not sum, of the transfers) ----
        wqk_sb = consts.tile([128, 4, 1024], bf)
        nc.sync.dma_start(out=wqk_sb, in_=wqk)
        bqk_sb = consts.tile([128, 8], f32)
        nc.sync.dma_start(out=bqk_sb, in_=bqk)
        wv_sb = consts.tile([128, 4, H * VD], bf)
        nc.gpsimd.dma_start(out=wv_sb, in_=wv)
        bvT_sb = consts.tile([128, 16], f32)
        nc.gpsimd.dma_start(out=bvT_sb, in_=bvT)
        ones_sb = consts.tile([128, 128], bf)
        nc.gpsimd.dma_start(out=ones_sb, in_=ones2d)
        # wp/bp are first needed at proj (~150us in): their DMAs are emitted
        # inside the b==0 loop to keep startup HBM bandwidth for wqk/x/bias
        wp_sb = consts.tile([128, 16, DIM], bf)
        bp_sb = consts.tile([1, DIM], bf)

        for b in range(BL):
            # ---- load xT[b]: [512, 784] -> [128, cc, 784] ----
            xT_sb = xpool.tile([128, 4, N], bf)
            nc.sync.dma_start(out=xT_sb, in_=xT[b])

            # ---- pass A: qkT[o, t] for all heads (o-chunks 0-3 = q, 4-7 = k) ----
            # cc-outer / itile-inner: each weight chunk loaded once, used twice
            qk_sb = qkpool.tile([128, 8, N], bf)
            for oc in range(8):
                ps = psB.tile([128, 1024], f32, tag="B", name=f"qkps{oc}")
                for cc in range(4):
                    for (i0, isz) in ITILES:
                        nc.tensor.matmul(
                            ps[:, i0:i0 + isz],
                            lhsT=wqk_sb[:, cc, oc * 128:(oc + 1) * 128],
                            rhs=xT_sb[:, cc, i0:i0 + isz],
                            start=(cc == 0),
                            stop=(cc == 3),
                        )
                nc.vector.tensor_scalar_add(
                    out=qk_sb[:, oc, :], in0=ps[:, :N], scalar1=bqk_sb[:, oc:oc + 1],
                )

            # ---- pass B: v[t, h*256+d] (no bias: it passes through softmax) ----
            # emitted as 14 deferred waves interleaved into hp0's scores loop
            v_sb = vpool.tile([128, 7, H * VD], bf)

            def make_v_wave(tc_i, t0, tsz, half):
                def emit():
                    pss = [psB.tile([128, 1024], f32, tag="B", name=f"vps{tc_i}_{half}_0"),
                           psB.tile([128, 1024], f32, tag="B", name=f"vps{tc_i}_{half}_1")]
                    for cc in range(4):
                        for w in range(2):
                            ovt = half * 2 + w
                            nc.tensor.matmul(
                                pss[w][:tsz, 0:512],
                                lhsT=xT_sb[:, cc, t0:t0 + tsz],
                                rhs=wv_sb[:, cc, ovt * 512:(ovt + 1) * 512],
                                start=(cc == 0),
                                stop=(cc == 3),
                            )
                    for w in range(2):
                        ovt = half * 2 + w
                        nc.vector.tensor_scalar_add(
                            out=v_sb[:tsz, tc_i, ovt * 512:(ovt + 1) * 512],
                            in0=pss[w][:tsz, 0:512], scalar1=0.0,
                        )
                return emit

            v_waves = [make_v_wave(tc_i, t0, tsz, half)
                       for tc_i, (t0, tsz) in enumerate(CHUNKS)
                       for half in range(2)]

            silu_sb = silupool.tile([128, 16, N], bf)

            if b == 0:
                nc.scalar.dma_start(out=wp_sb, in_=wp)
                nc.scalar.dma_start(out=bp_sb, in_=bp)

            # deferred den/av work groups for a finished head-pair
            def make_groups(hp, p_sb):
                state = {}

                def make_den(k):
                    def emit():
                        dps = psB.tile([128, 1024], f32, tag="B", name=f"denps{hp}_{k}")
                        for jc, (j0, jsz) in enumerate(CHUNKS):
                            for (i0, isz) in ITILES:
                                nc.tensor.matmul(
                                    dps[:, i0:i0 + isz],
                                    lhsT=ones_sb[:jsz, :],
                                    rhs=p_sb[:jsz, jc, k, i0:i0 + isz],
                                    start=(jc == 0),
                                    stop=(jc == 6),
                                )
                        rs = smalls.tile([128, N], f32, tag="rs", name=f"rs{hp}_{k}")
                        nc.vector.reciprocal_approx_fast(out=rs, in_=dps[:, :N])
                        state[k] = rs
                    return emit

                def make_av(k, dh):
                    h = 2 * hp + k

                    def emit():
                        aps = psB.tile([128, 1024], f32, tag="B", name=f"avps{hp}_{k}_{dh}")
                        for jc, (j0, jsz) in enumerate(CHUNKS):
                            for (i0, isz) in ITILES:
                                nc.tensor.matmul(
                                    aps[:, i0:i0 + isz],
                                    lhsT=v_sb[:jsz, jc, h * VD + dh * 128: h * VD + (dh + 1) * 128],
                                    rhs=p_sb[:jsz, jc, k, i0:i0 + isz],
                                    start=(jc == 0),
                                    stop=(jc == 6),
                                )
                        # normalize while evicting: silu input = avT * (1/den)
                        nc.vector.tensor_tensor(
                            out=silu_sb[:, h * 2 + dh, :], in0=aps[:, :N],
                            in1=state[k], op=MULT,
                        )
                    return emit

                return [make_den(0), make_den(1), make_av(0, 0),
                        make_av(0, 1), make_av(1, 0), make_av(1, 1)]

            # ---- head pairs: scores for hp interleaved with deferred work ----
            pending = []
            for hp in range(4):
                qoc, koc = hp, 4 + hp

                bias_k = []
                for k in range(2):
                    bt = biaspool.tile([128, 7, N], bf, tag="bt", name=f"bt{hp}_{k}")
                    nc.sync.dma_start(out=bt, in_=biast[2 * hp + k])
                    bias_k.append(bt)

                # per-jc filler: hp0 absorbs the v pass, later hps the deferred
                # den/av of hp-1, keeping PE busy while ACT runs exp
                filler = v_waves if hp == 0 else pending
                fper = (len(filler) + 6) // 7 if filler else 0

                p_sb = ppool.tile([128, 7, 2, N], bf)
                for jc, (j0, jsz) in enumerate(CHUNKS):
                    # deferred PE work first: it covers the exp/TT latency of
                    # the previous chunk (and of hp-1's tail at jc=0)
                    for g in filler[jc * fper:(jc + 1) * fper]:
                        g()
                    # one 4-bank tile: even head at col 0, odd head at col 1024;
                    # the e/o matmuls are issued back-to-back into disjoint
                    # 64-row groups and execute concurrently
                    pso = psA.tile([128, 2048], f32, tag="A", name=f"sc{hp}_{jc}")
                    for (i0, isz) in ITILES:
                        nc.tensor.matmul(
                            pso[:jsz, i0:i0 + isz],
                            lhsT=qk_sb[0:64, koc, j0:j0 + jsz],
                            rhs=qk_sb[0:64, qoc, i0:i0 + isz],
                            start=True, stop=True,
                        )
                        nc.tensor.matmul(
                            pso[:jsz, 1024 + i0:1024 + i0 + isz],
                            lhsT=qk_sb[64:128, koc, j0:j0 + jsz],
                            rhs=qk_sb[64:128, qoc, i0:i0 + isz],
                            start=True, stop=True,
                        )
                    for k in range(2):
                        # exp(S)*exp(bias) == exp(S+bias); biast holds exp(bias)
                        nc.scalar.activation(
                            out=p_sb[:jsz, jc, k, :],
                            in_=pso[:jsz, 1024 * k:1024 * k + N],
                            func=AF.Exp,
                        )
                        nc.vector.tensor_tensor(
                            out=p_sb[:jsz, jc, k, :], in0=p_sb[:jsz, jc, k, :],
                            in1=bias_k[k][:jsz, jc, :], op=MULT,
                        )

                pending = make_groups(hp, p_sb)

            # flush hp3's den/av; its PE time covers silu of earlier heads
            for g in pending:
                g()

            # ---- silu (with v-bias folded in per-partition) ----
            # the scale operand is a tile written only after the last norm
            # eviction: a pure scheduling gate that keeps every silu behind
            # all of this batch's exps (one Exp->Silu table switch per batch)
            gate = gatepool.tile([128, 1], f32)
            nc.vector.tensor_scalar(
                out=gate, in0=silu_sb[:, 15, 0:1], scalar1=0.0, scalar2=1.0,
                op0=MULT, op1=mybir.AluOpType.add,
            )
            for vc in range(16):
                nc.scalar.activation(
                    out=silu_sb[:, vc, :], in_=silu_sb[:, vc, :],
                    func=AF.Silu, bias=bvT_sb[:, vc:vc + 1], scale=gate[:, 0:1],
                )

            # ---- proj: lhsT = siluT chunks directly (no transposes) ----
            for tc_i, (t0, tsz) in enumerate(CHUNKS):
                psf = psB.tile([128, 1024], f32, tag="B", name=f"projps{tc_i}")
                nc.tensor.matmul(
                    psf[:tsz, 0:512],
                    lhsT=ones_sb[0:1, :tsz],
                    rhs=bp_sb[0:1, :],
                    start=True,
                    stop=False,
                )
                for vc in range(16):
                    nc.tensor.matmul(
                        psf[:tsz, 0:512],
                        lhsT=silu_sb[:, vc, t0:t0 + tsz],
                        rhs=wp_sb[:, vc, :],
                        start=False,
                        stop=(vc == 15),
                    )
                fo = fopool.tile([128, DIM], f32)
                nc.scalar.activation(out=fo[:tsz], in_=psf[:tsz, 0:512], func=AF.Copy)
                nc.scalar.dma_start(out=out[b, t0:t0 + tsz, :], in_=fo[:tsz])

    nc.finalize()
    return nc


def _prep(inputs):
    bf16 = ml_dtypes.bfloat16
    f32 = np.float32
    inputs = {k: np.asarray(v) for k, v in inputs.items()}

    s_qkv = (inputs["qkv_gamma"] / np.sqrt(inputs["qkv_var"] + EPS)).astype(f32)
    b_qkv = (inputs["qkv_beta"] - inputs["qkv_mean"] * s_qkv).astype(f32)
    w_fold = (inputs["qkv_w"] * s_qkv[:, None]).astype(f32)

    rows = np.arange((2 * KD + VD) * H).reshape(H, 2 * KD + VD)
    q_rows = rows[:, :KD].ravel()
    k_rows = rows[:, KD:2 * KD].ravel()
    v_rows = rows[:, 2 * KD:].ravel()

    wq = w_fold[q_rows] * SCALE
    bq = b_qkv[q_rows] * SCALE
    wk = w_fold[k_rows]
    bk = b_qkv[k_rows]
    wvm = w_fold[v_rows]
    bvm = b_qkv[v_rows]

    # wqk: [c, o] with o = [q(512), k(512)] -> [128, cc, 1024]
    wqkT = np.concatenate([wq, wk], axis=0).T.astype(bf16)          # [512, 1024]
    wqk_t = np.ascontiguousarray(wqkT.reshape(4, 128, 1024).transpose(1, 0, 2))
    bqk_t = np.concatenate([bq, bk]).reshape(8, 128).T.astype(f32)  # [128, 8]
    bqk_t = np.ascontiguousarray(bqk_t)

    wv_t = np.ascontiguousarray(
        wvm.T.astype(bf16).reshape(4, 128, H * VD).transpose(1, 0, 2)
    )
    # v bias in transposed layout: bvT[p, vc] = bvm[vc*128 + p]
    bvT_t = np.ascontiguousarray(bvm.astype(f32).reshape(16, 128).T)

    s_p = (inputs["proj_gamma"] / np.sqrt(inputs["proj_var"] + EPS)).astype(f32)
    b_p = (inputs["proj_beta"] - inputs["proj_mean"] * s_p).astype(f32)
    wp_fold = (inputs["proj_w"] * s_p[:, None]).astype(f32)          # [512, 2048]
    wp_t = np.ascontiguousarray(
        wp_fold.T.astype(bf16).reshape(16, 128, DIM).transpose(1, 0, 2)
    )
    bp_t = np.ascontiguousarray(b_p.astype(bf16)[None, :])

    bias_full = inputs["attention_biases"][:, inputs["bias_idxs"]].astype(f32)  # [H, N, N]
    biastp = np.zeros((H, NJP, N), dtype=bf16)
    biastp[:, :N, :] = np.exp(bias_full).astype(bf16)   # multiplicative form
    # SBUF-layout-matched: [H, partition, chunk, i] for fully linear DMA
    biast = np.ascontiguousarray(
        biastp.reshape(H, 7, 128, N).transpose(0, 2, 1, 3)
    )

    # x transposed and pre-chunked to the SBUF layout [b, partition, cc, t]
    xT = inputs["x"].transpose(0, 2, 1).astype(bf16)                 # [B, 512, 784]
    xT = np.ascontiguousarray(
        xT.reshape(B, 4, 128, N).transpose(0, 2, 1, 3)               # [B, 128, 4, 784]
    )

    shared = {
        "wqk": wqk_t, "wv": wv_t, "wp": wp_t, "bqk": bqk_t,
        "bvT": bvT_t, "bp": bp_t, "biast": biast,
        "ones2d": np.ones((128, 128), dtype=bf16),
    }
    in_maps = []
    for c in range(NCORES):
        m = dict(shared)
        m["xT"] = np.ascontiguousarray(xT[c * BL:(c + 1) * BL])
        in_maps.append(m)
    return in_maps


def kernel(trace=False, **inputs):
    from concourse import bass_utils

    if "nc" not in _CACHE:
        _CACHE["nc"] = _build_nc()
    nc = _CACHE["nc"]

    in_maps = _prep(inputs)
    res = bass_utils.run_bass_kernel_spmd(
        nc, in_maps, core_ids=list(range(NCORES)), trace=trace,
    )
    out = np.concatenate([r["out"] for r in res.results], axis=0)
    if trace:
        return out.astype(np.float32), res
    return out.astype(np.float32)


# revision 22
# speedup vs baseline: 1.0102x; 1.0102x over previous
# Fused attention block (LeViT-style) for Trainium2, 8 NeuronCores, data-parallel over batch.
#
# reference computation (B=16, N=784, DIM=512, H=8, KD=64, VD=256):
#   qkv = BN(x @ qkv_w.T); split q,k,v per head
#   attn = softmax(q @ k.T * KD**-0.5 + attention_biases[:, bias_idxs])
#   out  = BN(silu(attn @ v reshaped) @ proj_w.T)
#
# Strategy (v2 — transposed-AV formulation):
#  - batch-parallel: 2 batches per core, weights/bias tables replicated, no collectives
#  - BN folded into weights on host; softmax scale folded into q weights
#  - scores computed transposed (S^T[j,i]); bias table is symmetric so bias adds unchanged
#  - unstabilized softmax (scores empirically bounded ~|10|, exp safe in fp32)
#  - heads in even/odd pairs at PE row bases 0/64: the K=64 score matmuls of the
#    two heads share the 128-wide xbus and execute concurrently
#  - AV computed TRANSPOSED: avT[d,i] = V^T @ P^T with lhsT=V (weight-stationary,
#    free dim 512/272) -> result lands in the exact lhsT layout proj needs, so
#    no PE transposes and no PSUM->SBUF repack copies
#  - softmax denominators via an all-ones lhsT matmul chain accumulating over j
#    chunks; output is replicated across all 128 partitions -> normalization is
#    a single DVE multiply per (head, d-half), no broadcast step
#  - v bias passes through softmax unchanged (attn rows sum to 1), so it is
#    dropped from the v GEMM and folded into silu's per-partition bias operand
#  - two generic 2-bank PSUM pools serve every phase (scores/den/av/qk/v/proj)

import numpy as np
import ml_dtypes

B, N, DIM = 16, 784, 512
H, KD, VD = 8, 64, 256
RES = 28
EPS = 1e-5
SCALE = KD ** -0.5
NCORES = 8
BL = B // NCORES          # batches per core
NJP = 896                 # padded j extent (7 * 128)

# t/j chunking over N=784: six 128-chunks + one 16-chunk
CHUNKS = [(i * 128, min(128, N - i * 128)) for i in range((N + 127) // 128)]
ITILES = [(0, 512), (512, N - 512)]   # free-dim tiles for 784 (<=512 per PSUM bank)

_CACHE = {}


def _build_nc():
    from contextlib import ExitStack
    import concourse.bacc as bacc
    import concourse.tile as tile
    from concourse import mybir

    bf = mybir.dt.bfloat16
    f32 = mybir.dt.float32
    AF = mybir.ActivationFunctionType
    MULT = mybir.AluOpType.mult

    nc = bacc.Bacc("TRN2", target_bir_lowering=False, debug=False)

    xT = nc.dram_tensor("xT", [BL, 128, 4, N], bf, kind="ExternalInput").ap()
    wqk = nc.dram_tensor("wqk", [128, 4, 1024], bf, kind="ExternalInput").ap()
    wv = nc.dram_tensor("wv", [128, 4, H * VD], bf, kind="ExternalInput").ap()
    wp = nc.dram_tensor("wp", [128, 16, DIM], bf, kind="ExternalInput").ap()
    bqk = nc.dram_tensor("bqk", [128, 8], f32, kind="ExternalInput").ap()
    bvT = nc.dram_tensor("bvT", [128, 16], f32, kind="ExternalInput").ap()
    bp = nc.dram_tensor("bp", [1, DIM], bf, kind="ExternalInput").ap()
    biast = nc.dram_tensor("biast", [H, 128, 7, N], bf, kind="ExternalInput").ap()
    ones2d = nc.dram_tensor("ones2d", [128, 128], bf, kind="ExternalInput").ap()
    out = nc.dram_tensor("out", [BL, N, DIM], f32, kind="ExternalOutput").ap()

    import concourse.bass as bass

    with ExitStack() as ctx:
        tc = ctx.enter_context(tile.TileContext(nc))
        consts = ctx.enter_context(tc.tile_pool(name="consts", bufs=1))
        xpool = ctx.enter_context(tc.tile_pool(name="xpool", bufs=2))
        qkpool = ctx.enter_context(tc.tile_pool(name="qkpool", bufs=1))
        vpool = ctx.enter_context(tc.tile_pool(name="vpool", bufs=1))
        silupool = ctx.enter_context(tc.tile_pool(name="silupool", bufs=1))
        biaspool = ctx.enter_context(tc.tile_pool(name="biaspool", bufs=2))
        ppool = ctx.enter_context(tc.tile_pool(name="ppool", bufs=2))
        smalls = ctx.enter_context(tc.tile_pool(name="smalls", bufs=2))
        gatepool = ctx.enter_context(tc.tile_pool(name="gatepool", bufs=2))
        fopool = ctx.enter_context(tc.tile_pool(name="fopool", bufs=2))
        # scores PSUM: one 4-bank tile (even head at 0, odd head at 1024) so
        # the even/odd row-group-paired matmuls are never skewed by buffer
        # recycling; psB: 2 x 2-bank tiles for den/av/v accumulators
        psA = ctx.enter_context(tc.tile_pool(name="psA", bufs=1, space="PSUM"))
        psB = ctx.enter_context(tc.tile_pool(name="psB", bufs=2, space="PSUM"))

        # ---- constants (spread across engine DGE queues so the startup
        #      critical path is max, not sum, of the transfers) ----
        wqk_sb = consts.tile([128, 4, 1024], bf)
        nc.sync.dma_start(out=wqk_sb, in_=wqk)
        bqk_sb = consts.tile([128, 8], f32)
        nc.sync.dma_start(out=bqk_sb, in_=bqk)
        wv_sb = consts.tile([128, 4, H * VD], bf)
        nc.gpsimd.dma_start(out=wv_sb, in_=wv)
        bvT_sb = consts.tile([128, 16], f32)
        nc.gpsimd.dma_start(out=bvT_sb, in_=bvT)
        ones_sb = consts.tile([128, 128], bf)
        nc.gpsimd.dma_start(out=ones_sb, in_=ones2d)
        # wp/bp are first needed at proj (~150us in): their DMAs are emitted
        # inside the b==0 loop to keep startup HBM bandwidth for wqk/x/bias
        wp_sb = consts.tile([128, 16, DIM], bf)
        bp_sb = consts.tile([1, DIM], bf)

        for b in range(BL):
            # ---- load xT[b]: [512, 784] -> [128, cc, 784] ----
            xT_sb = xpool.tile([128, 4, N], bf)
            nc.sync.dma_start(out=xT_sb, in_=xT[b])

            # ---- pass A: qkT[o, t] for all heads (o-chunks 0-3 = q, 4-7 = k) ----
            # cc-outer / itile-inner: each weight chunk loaded once, used twice
            qk_sb = qkpool.tile([128, 8, N], bf)
            for oc in range(8):
                ps = psB.tile([128, 1024], f32, tag="B", name=f"qkps{oc}")
                for cc in range(4):
                    for (i0, isz) in ITILES:
                        nc.tensor.matmul(
                            ps[:, i0:i0 + isz],
                            lhsT=wqk_sb[:, cc, oc * 128:(oc + 1) * 128],
                            rhs=xT_sb[:, cc, i0:i0 + isz],
                            start=(cc == 0),
                            stop=(cc == 3),
                        )
                nc.vector.tensor_scalar_add(
                    out=qk_sb[:, oc, :], in0=ps[:, :N], scalar1=bqk_sb[:, oc:oc + 1],
                )

            # ---- pass B: v[t, h*256+d] (no bias: it passes through softmax) ----
            # emitted as 14 deferred waves interleaved into hp0's scores loop
            v_sb = vpool.tile([128, 7, H * VD], bf)

            def make_v_wave(tc_i, t0, tsz, half):
                def emit():
                    pss = [psB.tile([128, 1024], f32, tag="B", name=f"vps{tc_i}_{half}_0"),
                           psB.tile([128, 1024], f32, tag="B", name=f"vps{tc_i}_{half}_1")]
                    for cc in range(4):
                        for w in range(2):
                            ovt = half * 2 + w
                            nc.tensor.matmul(
                                pss[w][:tsz, 0:512],
                                lhsT=xT_sb[:, cc, t0:t0 + tsz],
                                rhs=wv_sb[:, cc, ovt * 512:(ovt + 1) * 512],
                                start=(cc == 0),
                                stop=(cc == 3),
                            )
                    for w in range(2):
                        ovt = half * 2 + w
                        nc.vector.tensor_scalar_add(
                            out=v_sb[:tsz, tc_i, ovt * 512:(ovt + 1) * 512],
                            in0=pss[w][:tsz, 0:512], scalar1=0.0,
                        )
                return emit

            v_waves = [make_v_wave(tc_i, t0, tsz, half)
                       for tc_i, (t0, tsz) in enumerate(CHUNKS)
                       for half in range(2)]

            silu_sb = silupool.tile([128, 16, N], bf)

            if b == 0:
                nc.scalar.dma_start(out=wp_sb, in_=wp)
                nc.scalar.dma_start(out=bp_sb, in_=bp)

            # deferred den/av work groups for a finished head-pair
            def make_groups(hp, p_sb):
                state = {}

                def make_den(k):
                    def emit():
                        dps = psB.tile([128, 1024], f32, tag="B", name=f"denps{hp}_{k}")
                        for jc, (j0, jsz) in enumerate(CHUNKS):
                            for (i0, isz) in ITILES:
                                nc.tensor.matmul(
                                    dps[:, i0:i0 + isz],
                                    lhsT=ones_sb[:jsz, :],
                                    rhs=p_sb[:jsz, jc, k, i0:i0 + isz],
                                    start=(jc == 0),
                                    stop=(jc == 6),
                                )
                        rs = smalls.tile([128, N], f32, tag="rs", name=f"rs{hp}_{k}")
                        nc.vector.reciprocal_approx_fast(out=rs, in_=dps[:, :N])
                        state[k] = rs
                    return emit

                def make_av(k, dh):
                    h = 2 * hp + k

                    def emit():
                        aps = psB.tile([128, 1024], f32, tag="B", name=f"avps{hp}_{k}_{dh}")
                        for jc, (j0, jsz) in enumerate(CHUNKS):
                            for (i0, isz) in ITILES:
                                nc.tensor.matmul(
                                    aps[:, i0:i0 + isz],
                                    lhsT=v_sb[:jsz, jc, h * VD + dh * 128: h * VD + (dh + 1) * 128],
                                    rhs=p_sb[:jsz, jc, k, i0:i0 + isz],
                                    start=(jc == 0),
                                    stop=(jc == 6),
                                )
                        # normalize while evicting: silu input = avT * (1/den)
                        nc.vector.tensor_tensor(
                            out=silu_sb[:, h * 2 + dh, :], in0=aps[:, :N],
                            in1=state[k], op=MULT,
                        )
                    return emit

                return [make_den(0), make_den(1), make_av(0, 0),
                        make_av(0, 1), make_av(1, 0), make_av(1, 1)]

            # ---- head pairs: scores for hp interleaved with deferred work ----
            pending = []
            for hp in range(4):
                qoc, koc = hp, 4 + hp

                bias_k = []
                for k in range(2):
                    bt = biaspool.tile([128, 7, N], bf, tag="bt", name=f"bt{hp}_{k}")
                    nc.sync.dma_start(out=bt, in_=biast[2 * hp + k])
                    bias_k.append(bt)

                # per-jc filler: hp0 absorbs the v pass, later hps the deferred
                # den/av of hp-1, keeping PE busy while ACT runs exp
                filler = v_waves if hp == 0 else pending
                fper = (len(filler) + 6) // 7 if filler else 0

                p_sb = ppool.tile([128, 7, 2, N], bf)
                for jc, (j0, jsz) in enumerate(CHUNKS):
                    # deferred PE work first: it covers the exp/TT latency of
                    # the previous chunk (and of hp-1's tail at jc=0)
                    for g in filler[jc * fper:(jc + 1) * fper]:
                        g()
                    # one 4-bank tile: even head at col 0, odd head at col 1024;
                    # the e/o matmuls are issued back-to-back into disjoint
                    # 64-row groups and execute concurrently
                    pso = psA.tile([128, 2048], f32, tag="A", name=f"sc{hp}_{jc}")
                    for (i0, isz) in ITILES:
                        nc.tensor.matmul(
                            pso[:jsz, i0:i0 + isz],
                            lhsT=qk_sb[0:64, koc, j0:j0 + jsz],
                            rhs=qk_sb[0:64, qoc, i0:i0 + isz],
                            start=True, stop=True,
                        )
                        nc.tensor.matmul(
                            pso[:jsz, 1024 + i0:1024 + i0 + isz],
                            lhsT=qk_sb[64:128, koc, j0:j0 + jsz],
                            rhs=qk_sb[64:128, qoc, i0:i0 + isz],
                            start=True, stop=True,
                        )
                    # exp(S)*exp(bias) == exp(S+bias); biast holds exp(bias).
                    # both heads in one ACT op (saves the ~290ns fixed cost)
                    psv = pso.rearrange("p (k n) -> p k n", n=1024)
                    nc.scalar.activation(
                        out=p_sb[:jsz, jc, :, :],
                        in_=psv[:jsz, :, :N],
                        func=AF.Exp,
                    )
                    for k in range(2):
                        nc.vector.tensor_tensor(
                            out=p_sb[:jsz, jc, k, :], in0=p_sb[:jsz, jc, k, :],
                            in1=bias_k[k][:jsz, jc, :], op=MULT,
                        )

                pending = make_groups(hp, p_sb)

            # flush hp3's den/av; its PE time covers silu of earlier heads
            for g in pending:
                g()

            # ---- silu (with v-bias folded in per-partition) ----
            # the scale operand is a tile written only after the last norm
            # eviction: a pure scheduling gate that keeps every silu behind
            # all of this batch's exps (one Exp->Silu table switch per batch)
            gate = gatepool.tile([128, 1], f32)
            nc.vector.tensor_scalar(
                out=gate, in0=silu_sb[:, 15, 0:1], scalar1=0.0, scalar2=1.0,
                op0=MULT, op1=mybir.AluOpType.add,
            )
            for vc in range(16):
                nc.scalar.activation(
                    out=silu_sb[:, vc, :], in_=silu_sb[:, vc, :],
                    func=AF.Silu, bias=bvT_sb[:, vc:vc + 1], scale=gate[:, 0:1],
                )

            # ---- proj: lhsT = siluT chunks directly (no transposes) ----
            for tc_i, (t0, tsz) in enumerate(CHUNKS):
                psf = psB.tile([128, 1024], f32, tag="B", name=f"projps{tc_i}")
                nc.tensor.matmul(
                    psf[:tsz, 0:512],
                    lhsT=ones_sb[0:1, :tsz],
                    rhs=bp_sb[0:1, :],
                    start=True,
                    stop=False,
                )
                for vc in range(16):
                    nc.tensor.matmul(
                        psf[:tsz, 0:512],
                        lhsT=silu_sb[:, vc, t0:t0 + tsz],
                        rhs=wp_sb[:, vc, :],
                        start=False,
                        stop=(vc == 15),
                    )
                fo = fopool.tile([128, DIM], f32)
                nc.scalar.activation(out=fo[:tsz], in_=psf[:tsz, 0:512], func=AF.Copy)
                nc.scalar.dma_start(out=out[b, t0:t0 + tsz, :], in_=fo[:tsz])

    nc.finalize()
    return nc


def _prep(inputs):
    bf16 = ml_dtypes.bfloat16
    f32 = np.float32
    inputs = {k: np.asarray(v) for k, v in inputs.items()}

    s_qkv = (inputs["qkv_gamma"] / np.sqrt(inputs["qkv_var"] + EPS)).astype(f32)
    b_qkv = (inputs["qkv_beta"] - inputs["qkv_mean"] * s_qkv).astype(f32)
    w_fold = (inputs["qkv_w"] * s_qkv[:, None]).astype(f32)

    rows = np.arange((2 * KD + VD) * H).reshape(H, 2 * KD + VD)
    q_rows = rows[:, :KD].ravel()
    k_rows = rows[:, KD:2 * KD].ravel()
    v_rows = rows[:, 2 * KD:].ravel()

    wq = w_fold[q_rows] * SCALE
    bq = b_qkv[q_rows] * SCALE
    wk = w_fold[k_rows]
    bk = b_qkv[k_rows]
    wvm = w_fold[v_rows]
    bvm = b_qkv[v_rows]

    # wqk: [c, o] with o = [q(512), k(512)] -> [128, cc, 1024]
    wqkT = np.concatenate([wq, wk], axis=0).T.astype(bf16)          # [512, 1024]
    wqk_t = np.ascontiguousarray(wqkT.reshape(4, 128, 1024).transpose(1, 0, 2))
    bqk_t = np.concatenate([bq, bk]).reshape(8, 128).T.astype(f32)  # [128, 8]
    bqk_t = np.ascontiguousarray(bqk_t)

    wv_t = np.ascontiguousarray(
        wvm.T.astype(bf16).reshape(4, 128, H * VD).transpose(1, 0, 2)
    )
    # v bias in transposed layout: bvT[p, vc] = bvm[vc*128 + p]
    bvT_t = np.ascontiguousarray(bvm.astype(f32).reshape(16, 128).T)

    s_p = (inputs["proj_gamma"] / np.sqrt(inputs["proj_var"] + EPS)).astype(f32)
    b_p = (inputs["proj_beta"] - inputs["proj_mean"] * s_p).astype(f32)
    wp_fold = (inputs["proj_w"] * s_p[:, None]).astype(f32)          # [512, 2048]
    wp_t = np.ascontiguousarray(
        wp_fold.T.astype(bf16).reshape(16, 128, DIM).transpose(1, 0, 2)
    )
    bp_t = np.ascontiguousarray(b_p.astype(bf16)[None, :])

    bias_full = inputs["attention_biases"][:, inputs["bias_idxs"]].astype(f32)  # [H, N, N]
    biastp = np.zeros((H, NJP, N), dtype=bf16)
    biastp[:, :N, :] = np.exp(bias_full).astype(bf16)   # multiplicative form
    # SBUF-layout-matched: [H, partition, chunk, i] for fully linear DMA
    biast = np.ascontiguousarray(
        biastp.reshape(H, 7, 128, N).transpose(0, 2, 1, 3)
    )

    # x transposed and pre-chunked to the SBUF layout [b, partition, cc, t]
    xT = inputs["x"].transpose(0, 2, 1).astype(bf16)                 # [B, 512, 784]
    xT = np.ascontiguousarray(
        xT.reshape(B, 4, 128, N).transpose(0, 2, 1, 3)               # [B, 128, 4, 784]
    )

    shared = {
        "wqk": wqk_t, "wv": wv_t, "wp": wp_t, "bqk": bqk_t,
        "bvT": bvT_t, "bp": bp_t, "biast": biast,
        "ones2d": np.ones((128, 128), dtype=bf16),
    }
    in_maps = []
    for c in range(NCORES):
        m = dict(shared)
        m["xT"] = np.ascontiguousarray(xT[c * BL:(c + 1) * BL])
        in_maps.append(m)
    return in_maps


def kernel(trace=False, **inputs):
    from concourse import bass_utils

    if "nc" not in _CACHE:
        _CACHE["nc"] = _build_nc()
    nc = _CACHE["nc"]

    in_maps = _prep(inputs)
    res = bass_utils.run_bass_kernel_spmd(
        nc, in_maps, core_ids=list(range(NCORES)), trace=trace,
    )
    out = np.concatenate([r["out"] for r in res.results], axis=0)
    if trace:
        return out.astype(np.float32), res
    return out.astype(np.float32)
